# revision 42
# baseline (speedup 1.0000x reference)
"""BitLinear kernel for Trainium2, 8 NeuronCores, column-parallel.

y[t, o] = sum_i x[t, i] * sign(W[o, i]) * scale[o]
  x: [8192, 4096] f32 (replicated), W: [16384, 4096] f32, scale: [16384] f32
  Each core owns OUT_F/8 = 2048 output features (column parallel).

v8: PE floor is 96 matmuls/tile x 216ns = 20.7us/tile (16 f16 + 8 fp8
DoubleRow subtile-pairs per 512-out band; the 16/16 f16-fp8 split is
error-optimal at rel_err 1.87e-2 vs the 1.95e-2 gate).  v6 measured
1.731ms, v7 1.570ms.  v7's remaining overhead was warm-phase PE idle
(121us: W bands landed at ~44/110/160/239us because W chunks shared the
gpsimd queue with x casts) plus a warm->steady transition stall (the
xta pool wait blocked the ACT queue).  v8:
  - W rides the SP queue EXCLUSIVELY as 64 raw-f32 [128,1024] units
    (plain 4KB-row DMAs, ~2.7us each) -> all 4 bands land by ~180us.
    gpsimd carries only x casts (all 10 warm tiles by ~110us).
  - sign conversion f32->+-1 f16 split between ACT (Sign activation)
    and DVE (u32 bit-trick (w&0x80000000)^0x3F800000 + f16 copy);
    PE transposes 8-subtile banks into PSUM f16; DVE copies PSUM->B
    fp8.  W never touches the packet-rate-limited XBAR.
  - x transposes split into two half-tile XBARs (subtiles 0-15 f16 for
    the f16 matmuls, 16-31 feeding the DVE fp8 cast); LOOK=3, WARM=10.
  - y emitted to SBUF as f16 (adds ~3e-4 rel err in quadrature) and
    stored as f16; kernel() casts back to f32 on host.  Steady tiles
    store one [128,2048] DMA (4KB rows) on SP; warm tiles store
    per-band (early tiles on gpsimd after the x casts, late on SP).
  - warm emission order comes from a static event simulation so no
    in-order engine queue blocks on a not-yet-ready dependency.
Engines: gpsimd=x casts + early warm y, SP=W raw loads + y stores,
ACT=XBARs + half the W signs, DVE=other half W signs + B copies + x fp8
casts + y emits, PE=W transposes + matmuls.
Scale: reference pins scale=ones, so the fast variant bakes sign into
B (+-1 fp8 exact) and skips scaling; kernel() host-checks scale and
falls back to a scaled-multiply DVE variant otherwise.
"""

import os
import sys

for _p in ("/opt/trn_rl_repo",):
    if _p not in sys.path and os.path.isdir(_p):
        sys.path.append(_p)

import numpy as np
import concourse.bacc as bacc
import concourse.mybir as mybir
from concourse.tile import TileContext
from concourse.masks import make_identity
from concourse.bass_utils import run_bass_kernel_spmd

TOKENS, IN_F, OUT_F, NCORES = 8192, 4096, 16384, 8
O_SH = OUT_F // NCORES  # 2048 out features per core
P = 128
KT = IN_F // P          # 32 k-subtiles
MT = TOKENS // P        # 64 token tiles
NBAND = 4               # 4 output bands of 512
BAND = O_SH // NBAND    # 512
F8SUB = 16              # trailing k-subtiles per band in fp8 DoubleRow
WARM = 10               # tiles emitted by the warm scheduler
PETILES = 5             # warm tiles transposed on the PE (rest via XBAR)
LOOK = 3                # steady-state lookahead (tiles)
NCHUNK = 32             # W chunks: (o-block 0..15) x (k-half 0..1), 1MB f32

f32, f16, u16, u32 = (
    mybir.dt.float32,
    mybir.dt.float16,
    mybir.dt.uint16,
    mybir.dt.uint32,
)
f8 = mybir.dt.float8e4
AF = mybir.ActivationFunctionType

_CACHE = {}
last_result = None


def build(apply_scale: bool):
    nc = bacc.Bacc("TRN2", target_bir_lowering=False, debug=False)
    x = nc.dram_tensor("x", [TOKENS, IN_F], f32, kind="ExternalInput").ap()
    w = nc.dram_tensor("weight", [O_SH, IN_F], f32, kind="ExternalInput").ap()
    scale = nc.dram_tensor("scale", [O_SH], f32, kind="ExternalInput").ap()
    y = nc.dram_tensor("y", [TOKENS, O_SH], f16, kind="ExternalOutput").ap()

    xta_bufs = 11 if not apply_scale else 8
    x8_bufs = 10
    yq_bufs = 10 if not apply_scale else 5

    with TileContext(nc) as tc:
        with (
            tc.tile_pool(name="const", bufs=1) as cpool,
            tc.tile_pool(name="bres", bufs=1) as bpool,
            tc.tile_pool(name="wraw", bufs=2) as wrawpool,
            tc.tile_pool(name="wsg", bufs=2) as wsgpool,
            tc.tile_pool(name="wq0", bufs=2) as wq0pool,
            tc.tile_pool(name="xstage", bufs=2) as xpool,
            tc.tile_pool(name="xta", bufs=xta_bufs) as xtapool,
            tc.tile_pool(name="xtb", bufs=2) as xtbpool,
            tc.tile_pool(name="x8p", bufs=x8_bufs) as x8pool,
            tc.tile_pool(name="yst", bufs=2) as ypool,
            tc.tile_pool(name="yq", bufs=yq_bufs) as yqpool,
            tc.tile_pool(name="mmps", bufs=6, space="PSUM") as mmps,
            tc.tile_pool(name="tpps", bufs=2, space="PSUM") as tpps,
        ):
            ident = cpool.tile([P, P], f16, tag="ident")
            make_identity(nc, ident)

            scale_bc = None
            if apply_scale:
                scale_p0 = cpool.tile([1, O_SH], f32, tag="scale_p0")
                nc.sync.dma_start(
                    scale_p0[:], scale.rearrange("(a o) -> a o", a=1)
                )
                scale_bc = cpool.tile([P, O_SH], f32, tag="scale_bc")
                nc.gpsimd.partition_broadcast(scale_bc[:], scale_p0[:])

            B = bpool.tile([P, KT, O_SH], f8, tag="B")

            # ---------- op emitters (called in simulated order) ----------
            wraw = {}     # unit -> raw f32 tile
            wsg = {}      # unit -> sign-converted f16 tile
            tpt = {}      # unit -> PSUM f16 transpose tile
            xcs, xtas, xtbs, x8s = {}, {}, {}, {}
            psb = {}      # (t, band) -> PSUM accumulation tile
            ystw = {}     # warm (t, band) -> yq tile

            def e_wload(c, on_q0):
                # chunk = [128 o, 2048 k]: 8KB-contiguous DRAM rows
                ot, kc = c >> 1, c & 1
                sl = w[ot * P : (ot + 1) * P, kc * 2048 : (kc + 1) * 2048]
                if on_q0:
                    t = wq0pool.tile([P, 2048], f16, tag="wq0")
                    nc.gpsimd.dma_start(t[:], sl)  # casting DMA
                    wsg[c] = t
                else:
                    t = wrawpool.tile([P, 2048], f32, tag="wraw")
                    nc.sync.dma_start(t[:], sl)
                    wraw[c] = t

            def e_conv(c, on_q0):
                # sign -> +-1 f16: SP chunks via ACT Sign (f32 source),
                # q0 cast chunks via DVE u16 bit-trick in place
                if on_q0:
                    t = wsg[c]
                    nc.vector.tensor_scalar(
                        t[:].bitcast(u16),
                        t[:].bitcast(u16),
                        0x8000,
                        0x3C00,
                        mybir.AluOpType.bitwise_and,
                        mybir.AluOpType.bitwise_xor,
                    )
                else:
                    t = wsgpool.tile([P, 2048], f16, tag="wsg")
                    nc.scalar.activation(t[:], wraw.pop(c)[:], AF.Sign)
                    wsg[c] = t

            def e_tbank(c, h):
                # transpose 8 of the chunk's k-subtiles into one PSUM bank
                t = wsg[c]
                tp = tpps.tile([P, 1024], f16, tag="tp")
                for j in range(8):
                    nc.tensor.transpose(
                        tp[:, j * P : (j + 1) * P],
                        t[:, (h * 8 + j) * P : (h * 8 + j + 1) * P],
                        ident[:],
                    )
                tpt[(c, h)] = tp

            def e_bcopy(c, h):
                ot, kc = c >> 1, c & 1
                k0 = kc * 16 + h * 8
                nc.vector.tensor_copy(
                    B[:, k0 : k0 + 8, ot * P : (ot + 1) * P],
                    tpt.pop((c, h))[:].rearrange("p (a b) -> p a b", a=8),
                )

            def e_xcast(t):
                xc = xpool.tile([P, IN_F], f16, tag="xc")
                nc.gpsimd.dma_start(xc[:], x[t * P : (t + 1) * P, :])
                xcs[t] = xc

            tpx = {}  # (t, g) -> PSUM transpose tile for PE-path x tiles

            def e_txbank(t, g):
                xc = xcs[t]
                tp = tpps.tile([P, 1024], f16, tag="tp")
                for j in range(8):
                    nc.tensor.transpose(
                        tp[:, j * P : (j + 1) * P],
                        xc[:, (g * 8 + j) * P : (g * 8 + j + 1) * P],
                        ident[:],
                    )
                tpx[(t, g)] = tp

            def e_txcopy(t, g):
                src = tpx.pop((t, g))[:].rearrange("p (a b) -> p a b", a=8)
                if g == 0:
                    xtas[t] = xtapool.tile(
                        [P, KT - F8SUB, P], f16, name=f"xtp{t}", tag="xta"
                    )
                if g == 2:
                    x8s[t] = x8pool.tile(
                        [P, F8SUB, P], f8, name=f"x8p{t}", tag="x8"
                    )
                if g < 2:
                    nc.vector.tensor_copy(
                        xtas[t][:, g * 8 : (g + 1) * 8, :], src
                    )
                else:
                    nc.vector.tensor_copy(
                        x8s[t][:, (g - 2) * 8 : (g - 1) * 8, :], src
                    )

            def e_xbarA(t):
                xta = xtapool.tile([P, KT - F8SUB, P], f16, tag="xta")
                nc.scalar.dma_start_transpose(xta[:], xcs[t][:, 0:2048])
                xtas[t] = xta

            def e_xbarB(t):
                xtb = xtbpool.tile([P, F8SUB, P], f16, tag="xtb")
                nc.scalar.dma_start_transpose(xtb[:], xcs[t][:, 2048:IN_F])
                xtbs[t] = xtb

            def e_x8(t):
                x8 = x8pool.tile([P, F8SUB, P], f8, tag="x8")
                nc.vector.tensor_copy(x8[:], xtbs.pop(t)[:])
                x8s[t] = x8

            def e_mm(t, b):
                n0 = b * BAND
                ps = mmps.tile([P, BAND], f32, tag="ps")
                xta, x8 = xtas[t], x8s[t]
                for k in range(KT - F8SUB):
                    nc.tensor.matmul(
                        ps[:],
                        xta[:, k, :],
                        B[:, k, n0 : n0 + BAND],
                        start=(k == 0),
                        stop=False,
                    )
                for j in range(F8SUB // 2):
                    k0 = KT - F8SUB + 2 * j
                    nc.tensor.matmul(
                        ps[:],
                        x8[:, 2 * j : 2 * j + 2, :],
                        B[:, k0 : k0 + 2, n0 : n0 + BAND],
                        start=False,
                        stop=(j == F8SUB // 2 - 1),
                        perf_mode=mybir.MatmulPerfMode.DoubleRow,
                    )
                psb[(t, b)] = ps

            def e_yemit(t, b, dst, n0_dst):
                ps = psb.pop((t, b))
                if apply_scale:
                    nc.vector.tensor_tensor(
                        dst[:, n0_dst : n0_dst + BAND],
                        ps[:],
                        scale_bc[:, b * BAND : (b + 1) * BAND],
                        mybir.AluOpType.mult,
                    )
                else:
                    nc.vector.tensor_copy(dst[:, n0_dst : n0_dst + BAND], ps[:])

            # ---------- warm phase: static event simulation ----------
            # The sim models every tile pool as a FIFO resource: an
            # allocating op waits for the readers of the alloc bufs-ago.
            # Emission in sim order therefore always yields an acyclic
            # wait graph (no cross-queue deadlock).
            # costs (us): calibrated against the v10 trace
            TSPW, TQ0X, TQ0W = 6.5, 12.0, 7.0
            TXB, TMM, TTB = 7.5, 5.2, 0.55
            TBC, TX8 = 0.9, 1.6
            TCV_ACT, TCV_DVE, TYE, TYD = 1.8, 0.6, 0.5, 1.4

            POOLS = {
                "wraw": 2, "wsg": 2, "wq0": 2, "tpps": 2, "xta": xta_bufs,
                "xtb": 2, "x8p": x8_bufs, "mmps": 6, "yq": yq_bufs,
                "xc": 2,
            }
            ops, readers = {}, {}

            def add(key, eng, cost, deps, pri, pool=None):
                ops[key] = (eng, cost, deps, pri, pool)

            # W chunks: o-block 4b+3 of each band cast-loads on gpsimd
            # between the warm x casts; the other 24 stream raw on SP.
            q0_chunks = set()
            for b in range(NBAND):
                for kc in range(2):
                    q0_chunks.add(((4 * b + 3) << 1) | kc)
            q0_chunks.add(2 << 1)  # one extra band-0 chunk for a fast start
            sp_plan = []
            for b in range(NBAND):
                for oi in range(4):
                    for kc in range(2):
                        c = ((4 * b + oi) << 1) | kc
                        if c not in q0_chunks:
                            sp_plan.append(c)
            for i, c in enumerate(sp_plan):
                add(("W", c), "sp", TSPW, [], 0 + i * 1e-4, "wraw")
                readers[("W", c)] = [("cv", c)]
            # The x pipeline is staged through WARM+LOOK so the steady
            # boundary tiles are ready before the PE needs them.
            NSTAGE = WARM + LOOK
            for t in range(NSTAGE):
                add(("x", t), "q0", TQ0X, [], 4.0 * t, "xc")
                readers[("x", t)] = (
                    [("xa", t), ("xb", t)] if t >= PETILES
                    else [("tx", t, g) for g in range(4)]
                )
            for i, c in enumerate(sorted(q0_chunks)):
                add(("W", c), "q0", TQ0W, [], 4.0 * (i // 2) + 1 + (i % 2) * 0.1,
                    "wq0")
                # cast chunk converts in place; last readers are the tbanks
                readers[("W", c)] = [("tb", c, 0), ("tb", c, 1)]
            for c in range(NCHUNK):
                on_q0 = c in q0_chunks
                add(("cv", c), "dve" if on_q0 else "act",
                    TCV_DVE if on_q0 else TCV_ACT, [("W", c)], 0.8,
                    None if on_q0 else "wsg")
                readers[("cv", c)] = [("tb", c, 0), ("tb", c, 1)]
                for h in range(2):
                    add(("tb", c, h), "pe", TTB, [("cv", c)], 0, "tpps")
                    readers[("tb", c, h)] = [("bc", c, h)]
                    add(("bc", c, h), "dve", TBC, [("tb", c, h)], 2)
            # first PETILES tiles transpose on the PE (fills warm PE idle
            # and keeps ACT free for W signs); later tiles use the XBAR
            # with just-in-time floors so they don't hog ACT early.
            ends, mmdep = {}, {}
            TXC2 = 1.1
            for t in range(PETILES):
                for g in range(4):
                    add(("tx", t, g), "pe", TTB, [("x", t)], 0.2, "tpps")
                    readers[("tx", t, g)] = [("xc2", t, g)]
                    pool = "xta" if g == 0 else ("x8p" if g == 2 else None)
                    add(("xc2", t, g), "dve", TXC2, [("tx", t, g)], 0.6, pool)
                readers[("xc2", t, 0)] = [("mm", t, b) for b in range(NBAND)]
                readers[("xc2", t, 2)] = [("mm", t, b) for b in range(NBAND)]
                mmdep[t] = [("xc2", t, g) for g in range(4)]
            for t in range(PETILES, NSTAGE):
                ends[("xfloor", t)] = 16.0 + 8.0 * t
                add(("xa", t), "act", TXB, [("x", t), ("xfloor", t)],
                    0 + t * 1e-4, "xta")
                readers[("xa", t)] = [("mm", t, b) for b in range(NBAND)]
                add(("xb", t), "act", TXB, [("xa", t)], 0 + t * 1e-4, "xtb")
                readers[("xb", t)] = [("x8", t)]
                add(("x8", t), "dve", TX8, [("xb", t)], 0.5, "x8p")
                readers[("x8", t)] = [("mm", t, b) for b in range(NBAND)]
                if t < WARM:
                    mmdep[t] = [("xa", t), ("x8", t)]
            bdep = {
                b: [("bc", ((4 * b + oi) << 1) | kc, h)
                    for oi in range(4) for kc in range(2) for h in range(2)]
                for b in range(NBAND)
            }
            for t in range(WARM):
                for b in range(NBAND):
                    add(("mm", t, b), "pe", TMM,
                        mmdep[t] + bdep[b], 1, "mmps")
                    readers[("mm", t, b)] = [("ye", t, b)]
                    add(("ye", t, b), "dve", TYE, [("mm", t, b)], 2, "yq")
                    readers[("ye", t, b)] = [("yd", t, b)]
                    # early warm tiles store y on gpsimd (free after x
                    # casts), late ones on SP (free after W)
                    add(("yd", t, b), "q0" if t < 6 else "sp", TYD,
                        [("ye", t, b)], 3)

            engclk = {"dve": 0.0, "act": 0.0, "pe": 0.0, "sp": 0.0, "q0": 0.0}
            fifo = {p: [] for p in POOLS}
            emitted = []
            remaining = dict(ops)
            while remaining:
                best = None
                for key, (eng, cost, deps, pri, pool) in remaining.items():
                    if any(d not in ends for d in deps):
                        continue
                    start = max([engclk[eng]] + [ends[d] for d in deps])
                    if pool is not None:
                        n = len(fifo[pool])
                        if n >= POOLS[pool]:
                            victim = fifo[pool][n - POOLS[pool]]
                            rd = readers.get(victim, [])
                            if any(r not in ends for r in rd):
                                continue
                            start = max([start] + [ends[r] for r in rd])
                    if best is None or (start, pri) < best[0]:
                        best = ((start, pri), key)
                (start, _), key = best
                eng, cost, deps, pri, pool = remaining.pop(key)
                ends[key] = start + cost
                engclk[eng] = start + cost
                if pool is not None:
                    fifo[pool].append(key)
                emitted.append(key)

            # emit all warm ops (including DMA loads) in simulated order
            for k in emitted:
                if k[0] == "W":
                    e_wload(k[1], k[1] in q0_chunks)
                    continue
                if k[0] == "x":
                    e_xcast(k[1])
                    continue
                if k[0] == "cv":
                    e_conv(k[1], k[1] in q0_chunks)
                elif k[0] == "tx":
                    e_txbank(k[1], k[2])
                elif k[0] == "xc2":
                    e_txcopy(k[1], k[2])
                elif k[0] == "tb":
                    e_tbank(k[1], k[2])
                elif k[0] == "bc":
                    e_bcopy(k[1], k[2])
                elif k[0] == "xa":
                    e_xbarA(k[1])
                elif k[0] == "xb":
                    e_xbarB(k[1])
                elif k[0] == "x8":
                    e_x8(k[1])
                elif k[0] == "mm":
                    e_mm(k[1], k[2])
                elif k[0] == "ye":
                    t, b = k[1], k[2]
                    yq = yqpool.tile([P, BAND], f16, tag="yq")
                    ystw[(t, b)] = yq
                    e_yemit(t, b, yq, 0)
                elif k[0] == "yd":
                    t, b = k[1], k[2]
                    eng = nc.gpsimd if t < 6 else nc.sync
                    eng.dma_start(
                        y[t * P : (t + 1) * P, b * BAND : (b + 1) * BAND],
                        ystw.pop((t, b))[:],
                    )

            # ---------- steady phase (t10..12 already staged in warm) ----------
            for mt in range(WARM + LOOK, MT + LOOK):
                if mt < MT:
                    e_xcast(mt)
                    e_xbarA(mt)
                    e_xbarB(mt)
                    e_x8(mt)
                rt = mt - LOOK
                yst = ypool.tile([P, O_SH], f16, tag="yst")
                for b in range(NBAND):
                    e_mm(rt, b)
                    e_yemit(rt, b, yst, b * BAND)
                nc.sync.dma_start(y[rt * P : (rt + 1) * P, :], yst[:])

    nc.finalize()
    return nc


def _get_nc(apply_scale: bool):
    key = "scale" if apply_scale else "ones"
    if key not in _CACHE:
        _CACHE[key] = build(apply_scale)
    return _CACHE[key]


def kernel(x, weight, scale):
    global last_result
    x = np.ascontiguousarray(np.asarray(x, dtype=np.float32))
    weight = np.ascontiguousarray(np.asarray(weight, dtype=np.float32))
    scale = np.ascontiguousarray(np.asarray(scale, dtype=np.float32))
    apply_scale = not bool(np.all(scale == 1.0))
    nc = _get_nc(apply_scale)
    in_maps = [
        {
            "x": x,
            "weight": np.ascontiguousarray(weight[c * O_SH : (c + 1) * O_SH]),
            "scale": np.ascontiguousarray(scale[c * O_SH : (c + 1) * O_SH]),
        }
        for c in range(NCORES)
    ]
    res = run_bass_kernel_spmd(nc, in_maps, list(range(NCORES)))
    last_result = res
    return np.concatenate(
        [np.asarray(res.results[c]["y"]).astype(np.float32) for c in range(NCORES)],
        axis=1,
    )


if __name__ == "__main__":
    rng = np.random.default_rng(0)
    xv = rng.standard_normal((TOKENS, IN_F), dtype=np.float32)
    wv = rng.standard_normal((OUT_F, IN_F), dtype=np.float32)
    sv = np.ones(OUT_F, dtype=np.float32)
    yv = kernel(xv, wv, sv)
    print("out shape:", yv.shape, yv.dtype)


# revision 43
# speedup vs baseline: 1.1837x; 1.1837x over previous
"""BitLinear kernel for Trainium2, 8 NeuronCores, column-parallel.

y[t, o] = sum_i x[t, i] * sign(W[o, i]) * scale[o]
  x: [8192, 4096] f32 (replicated), W: [16384, 4096] f32, scale: [16384] f32
  Each core owns OUT_F/8 = 2048 output features (column parallel).

v8: PE floor is 96 matmuls/tile x 216ns = 20.7us/tile (16 f16 + 8 fp8
DoubleRow subtile-pairs per 512-out band; the 16/16 f16-fp8 split is
error-optimal at rel_err 1.87e-2 vs the 1.95e-2 gate).  v6 measured
1.731ms, v7 1.570ms.  v7's remaining overhead was warm-phase PE idle
(121us: W bands landed at ~44/110/160/239us because W chunks shared the
gpsimd queue with x casts) plus a warm->steady transition stall (the
xta pool wait blocked the ACT queue).  v8:
  - W rides the SP queue EXCLUSIVELY as 64 raw-f32 [128,1024] units
    (plain 4KB-row DMAs, ~2.7us each) -> all 4 bands land by ~180us.
    gpsimd carries only x casts (all 10 warm tiles by ~110us).
  - sign conversion f32->+-1 f16 split between ACT (Sign activation)
    and DVE (u32 bit-trick (w&0x80000000)^0x3F800000 + f16 copy);
    PE transposes 8-subtile banks into PSUM f16; DVE copies PSUM->B
    fp8.  W never touches the packet-rate-limited XBAR.
  - x transposes split into two half-tile XBARs (subtiles 0-15 f16 for
    the f16 matmuls, 16-31 feeding the DVE fp8 cast); LOOK=3, WARM=10.
  - y emitted to SBUF as f16 (adds ~3e-4 rel err in quadrature) and
    stored as f16; kernel() casts back to f32 on host.  Steady tiles
    store one [128,2048] DMA (4KB rows) on SP; warm tiles store
    per-band (early tiles on gpsimd after the x casts, late on SP).
  - warm emission order comes from a static event simulation so no
    in-order engine queue blocks on a not-yet-ready dependency.
Engines: gpsimd=x casts + early warm y, SP=W raw loads + y stores,
ACT=XBARs + half the W signs, DVE=other half W signs + B copies + x fp8
casts + y emits, PE=W transposes + matmuls.
Scale: reference pins scale=ones, so the fast variant bakes sign into
B (+-1 fp8 exact) and skips scaling; kernel() host-checks scale and
falls back to a scaled-multiply DVE variant otherwise.
"""

import os
import sys

for _p in ("/opt/trn_rl_repo",):
    if _p not in sys.path and os.path.isdir(_p):
        sys.path.append(_p)

import numpy as np
import concourse.bacc as bacc
import concourse.mybir as mybir
from concourse.tile import TileContext
from concourse.masks import make_identity
from concourse.bass_utils import run_bass_kernel_spmd

TOKENS, IN_F, OUT_F, NCORES = 8192, 4096, 16384, 8
O_SH = OUT_F // NCORES  # 2048 out features per core
P = 128
KT = IN_F // P          # 32 k-subtiles
MT = TOKENS // P        # 64 token tiles
NBAND = 4               # 4 output bands of 512
BAND = O_SH // NBAND    # 512
F8SUB = 16              # trailing k-subtiles per band in fp8 DoubleRow
WARM = 10               # tiles emitted by the warm scheduler
PETILES = 5             # warm tiles transposed on the PE (rest via XBAR)
LOOK = 3                # steady-state lookahead (tiles)
NCHUNK = 32             # W chunks: (o-block 0..15) x (k-half 0..1), 1MB f32

f32, f16, u16, u32 = (
    mybir.dt.float32,
    mybir.dt.float16,
    mybir.dt.uint16,
    mybir.dt.uint32,
)
f8 = mybir.dt.float8e4
AF = mybir.ActivationFunctionType

_CACHE = {}
last_result = None


def build(apply_scale: bool):
    nc = bacc.Bacc("TRN2", target_bir_lowering=False, debug=False)
    x = nc.dram_tensor("x", [TOKENS, IN_F], f32, kind="ExternalInput").ap()
    w = nc.dram_tensor("weight", [O_SH, IN_F], f32, kind="ExternalInput").ap()
    scale = nc.dram_tensor("scale", [O_SH], f32, kind="ExternalInput").ap()
    y = nc.dram_tensor("y", [TOKENS, O_SH], f16, kind="ExternalOutput").ap()

    xta_bufs = 10 if not apply_scale else 8
    x8_bufs = 11
    yq_bufs = 10 if not apply_scale else 5

    with TileContext(nc) as tc:
        with (
            tc.tile_pool(name="const", bufs=1) as cpool,
            tc.tile_pool(name="bres", bufs=1) as bpool,
            tc.tile_pool(name="wraw", bufs=2) as wrawpool,
            tc.tile_pool(name="wsg", bufs=2) as wsgpool,
            tc.tile_pool(name="wq0", bufs=2) as wq0pool,
            tc.tile_pool(name="xstage", bufs=2) as xpool,
            tc.tile_pool(name="xta", bufs=xta_bufs) as xtapool,
            tc.tile_pool(name="xtb", bufs=3) as xtbpool,
            tc.tile_pool(name="x8p", bufs=x8_bufs) as x8pool,
            tc.tile_pool(name="yst", bufs=2) as ypool,
            tc.tile_pool(name="yq", bufs=yq_bufs) as yqpool,
            tc.tile_pool(name="mmps", bufs=6, space="PSUM") as mmps,
            tc.tile_pool(name="tpps", bufs=2, space="PSUM") as tpps,
        ):
            ident = cpool.tile([P, P], f16, tag="ident")
            make_identity(nc, ident)

            scale_bc = None
            if apply_scale:
                scale_p0 = cpool.tile([1, O_SH], f32, tag="scale_p0")
                nc.sync.dma_start(
                    scale_p0[:], scale.rearrange("(a o) -> a o", a=1)
                )
                scale_bc = cpool.tile([P, O_SH], f32, tag="scale_bc")
                nc.gpsimd.partition_broadcast(scale_bc[:], scale_p0[:])

            B = bpool.tile([P, KT, O_SH], f8, tag="B")

            # ---------- op emitters (called in simulated order) ----------
            wraw = {}     # unit -> raw f32 tile
            wsg = {}      # unit -> sign-converted f16 tile
            tpt = {}      # unit -> PSUM f16 transpose tile
            xcs, xtas, xtbs, x8s = {}, {}, {}, {}
            psb = {}      # (t, band) -> PSUM accumulation tile
            ystw = {}     # warm (t, band) -> yq tile

            def e_wload(c, on_q0):
                # chunk = [128 o, 2048 k]: 8KB-contiguous DRAM rows
                ot, kc = c >> 1, c & 1
                sl = w[ot * P : (ot + 1) * P, kc * 2048 : (kc + 1) * 2048]
                if on_q0:
                    t = wq0pool.tile([P, 2048], f16, tag="wq0")
                    nc.gpsimd.dma_start(t[:], sl)  # casting DMA
                    wsg[c] = t
                else:
                    t = wrawpool.tile([P, 2048], f32, tag="wraw")
                    nc.sync.dma_start(t[:], sl)
                    wraw[c] = t

            def e_conv(c, on_q0):
                # sign -> +-1 f16: SP chunks via ACT Sign (f32 source),
                # q0 cast chunks via DVE u16 bit-trick in place
                if on_q0:
                    t = wsg[c]
                    nc.vector.tensor_scalar(
                        t[:].bitcast(u16),
                        t[:].bitcast(u16),
                        0x8000,
                        0x3C00,
                        mybir.AluOpType.bitwise_and,
                        mybir.AluOpType.bitwise_xor,
                    )
                else:
                    t = wsgpool.tile([P, 2048], f16, tag="wsg")
                    nc.scalar.activation(t[:], wraw.pop(c)[:], AF.Sign)
                    wsg[c] = t

            def e_tbank(c, h):
                # transpose 8 of the chunk's k-subtiles into one PSUM bank
                t = wsg[c]
                tp = tpps.tile([P, 1024], f16, tag="tp")
                for j in range(8):
                    nc.tensor.transpose(
                        tp[:, j * P : (j + 1) * P],
                        t[:, (h * 8 + j) * P : (h * 8 + j + 1) * P],
                        ident[:],
                    )
                tpt[(c, h)] = tp

            def e_bcopy(c, h):
                ot, kc = c >> 1, c & 1
                k0 = kc * 16 + h * 8
                nc.vector.tensor_copy(
                    B[:, k0 : k0 + 8, ot * P : (ot + 1) * P],
                    tpt.pop((c, h))[:].rearrange("p (a b) -> p a b", a=8),
                )

            def e_xcast(t):
                xc = xpool.tile([P, IN_F], f16, tag="xc")
                nc.gpsimd.dma_start(xc[:], x[t * P : (t + 1) * P, :])
                xcs[t] = xc

            tpx = {}  # (t, g) -> PSUM transpose tile for PE-path x tiles

            def e_txbank(t, g):
                xc = xcs[t]
                tp = tpps.tile([P, 1024], f16, tag="tp")
                for j in range(8):
                    nc.tensor.transpose(
                        tp[:, j * P : (j + 1) * P],
                        xc[:, (g * 8 + j) * P : (g * 8 + j + 1) * P],
                        ident[:],
                    )
                tpx[(t, g)] = tp

            def e_txcopy(t, g):
                src = tpx.pop((t, g))[:].rearrange("p (a b) -> p a b", a=8)
                if g == 0:
                    xtas[t] = xtapool.tile(
                        [P, KT - F8SUB, P], f16, name=f"xtp{t}", tag="xta"
                    )
                if g == 2:
                    x8s[t] = x8pool.tile(
                        [P, F8SUB, P], f8, name=f"x8p{t}", tag="x8"
                    )
                if g < 2:
                    nc.vector.tensor_copy(
                        xtas[t][:, g * 8 : (g + 1) * 8, :], src
                    )
                else:
                    nc.vector.tensor_copy(
                        x8s[t][:, (g - 2) * 8 : (g - 1) * 8, :], src
                    )

            def e_xbarA(t):
                xta = xtapool.tile([P, KT - F8SUB, P], f16, tag="xta")
                nc.scalar.dma_start_transpose(xta[:], xcs[t][:, 0:2048])
                xtas[t] = xta

            def e_xbarB(t):
                xtb = xtbpool.tile([P, F8SUB, P], f16, tag="xtb")
                nc.scalar.dma_start_transpose(xtb[:], xcs[t][:, 2048:IN_F])
                xtbs[t] = xtb

            def e_x8(t):
                x8 = x8pool.tile([P, F8SUB, P], f8, tag="x8")
                nc.vector.tensor_copy(x8[:], xtbs.pop(t)[:])
                x8s[t] = x8

            def e_mm(t, b):
                n0 = b * BAND
                ps = mmps.tile([P, BAND], f32, tag="ps")
                xta, x8 = xtas[t], x8s[t]
                for k in range(KT - F8SUB):
                    nc.tensor.matmul(
                        ps[:],
                        xta[:, k, :],
                        B[:, k, n0 : n0 + BAND],
                        start=(k == 0),
                        stop=False,
                    )
                for j in range(F8SUB // 2):
                    k0 = KT - F8SUB + 2 * j
                    nc.tensor.matmul(
                        ps[:],
                        x8[:, 2 * j : 2 * j + 2, :],
                        B[:, k0 : k0 + 2, n0 : n0 + BAND],
                        start=False,
                        stop=(j == F8SUB // 2 - 1),
                        perf_mode=mybir.MatmulPerfMode.DoubleRow,
                    )
                psb[(t, b)] = ps

            def e_yemit(t, b, dst, n0_dst):
                ps = psb.pop((t, b))
                if apply_scale:
                    nc.vector.tensor_tensor(
                        dst[:, n0_dst : n0_dst + BAND],
                        ps[:],
                        scale_bc[:, b * BAND : (b + 1) * BAND],
                        mybir.AluOpType.mult,
                    )
                else:
                    nc.vector.tensor_copy(dst[:, n0_dst : n0_dst + BAND], ps[:])

            # ---------- warm phase: static event simulation ----------
            # The sim models every tile pool as a FIFO resource: an
            # allocating op waits for the readers of the alloc bufs-ago.
            # Emission in sim order therefore always yields an acyclic
            # wait graph (no cross-queue deadlock).
            # costs (us): calibrated against the v10 trace
            TSPW, TQ0X, TQ0W = 6.5, 12.0, 7.0
            TXB, TMM, TTB = 7.5, 5.2, 0.55
            TBC, TX8 = 0.9, 1.6
            TCV_ACT, TCV_DVE, TYE, TYD = 1.8, 0.6, 0.5, 1.4

            POOLS = {
                "wraw": 2, "wsg": 2, "wq0": 2, "tpps": 2, "xta": xta_bufs,
                "xtb": 3, "x8p": x8_bufs, "mmps": 6, "yq": yq_bufs,
                "xc": 2,
            }
            ops, readers = {}, {}

            def add(key, eng, cost, deps, pri, pool=None):
                ops[key] = (eng, cost, deps, pri, pool)

            # W chunks: o-block 4b+3 of each band cast-loads on gpsimd
            # between the warm x casts; the other 24 stream raw on SP.
            q0_chunks = set()
            for b in range(NBAND):
                for kc in range(2):
                    q0_chunks.add(((4 * b + 3) << 1) | kc)
            sp_plan = []
            for b in range(NBAND):
                for oi in range(3):
                    for kc in range(2):
                        sp_plan.append(((4 * b + oi) << 1) | kc)
            for i, c in enumerate(sp_plan):
                add(("W", c), "sp", TSPW, [], 0 + i * 1e-4, "wraw")
                readers[("W", c)] = [("cv", c)]
            # The x pipeline is staged through WARM+LOOK so the steady
            # boundary tiles are ready before the PE needs them.
            NSTAGE = WARM + LOOK
            for t in range(NSTAGE):
                add(("x", t), "q0", TQ0X, [], 4.0 * t, "xc")
                readers[("x", t)] = (
                    [("xa", t), ("xb", t)] if t >= PETILES
                    else [("tx", t, g) for g in range(4)]
                )
            for i, c in enumerate(sorted(q0_chunks)):
                add(("W", c), "q0", TQ0W, [], 4.0 * (i // 2) + 1 + (i % 2) * 0.1,
                    "wq0")
                # cast chunk converts in place; last readers are the tbanks
                readers[("W", c)] = [("tb", c, 0), ("tb", c, 1)]
            for c in range(NCHUNK):
                on_q0 = c in q0_chunks
                add(("cv", c), "dve" if on_q0 else "act",
                    TCV_DVE if on_q0 else TCV_ACT, [("W", c)], 0.8,
                    None if on_q0 else "wsg")
                readers[("cv", c)] = [("tb", c, 0), ("tb", c, 1)]
                for h in range(2):
                    add(("tb", c, h), "pe", TTB, [("cv", c)], 0, "tpps")
                    readers[("tb", c, h)] = [("bc", c, h)]
                    add(("bc", c, h), "dve", TBC, [("tb", c, h)], 2)
            # first PETILES tiles transpose on the PE (fills warm PE idle
            # and keeps ACT free for W signs); later tiles use the XBAR
            # with just-in-time floors so they don't hog ACT early.
            ends, mmdep = {}, {}
            TXC2 = 1.1
            for t in range(PETILES):
                for g in range(4):
                    add(("tx", t, g), "pe", TTB, [("x", t)], 0.2, "tpps")
                    readers[("tx", t, g)] = [("xc2", t, g)]
                    pool = "xta" if g == 0 else ("x8p" if g == 2 else None)
                    add(("xc2", t, g), "dve", TXC2, [("tx", t, g)], 0.6, pool)
                readers[("xc2", t, 0)] = [("mm", t, b) for b in range(NBAND)]
                readers[("xc2", t, 2)] = [("mm", t, b) for b in range(NBAND)]
                mmdep[t] = [("xc2", t, g) for g in range(4)]
            for t in range(PETILES, NSTAGE):
                ends[("xfloor", t)] = 16.0 + 8.0 * t
                add(("xa", t), "act", TXB, [("x", t), ("xfloor", t)],
                    0 + t * 1e-4, "xta")
                readers[("xa", t)] = [("mm", t, b) for b in range(NBAND)]
                add(("xb", t), "act", TXB, [("xa", t)], 0 + t * 1e-4, "xtb")
                readers[("xb", t)] = [("x8", t)]
                add(("x8", t), "dve", TX8, [("xb", t)], 0.5, "x8p")
                readers[("x8", t)] = [("mm", t, b) for b in range(NBAND)]
                if t < WARM:
                    mmdep[t] = [("xa", t), ("x8", t)]
            bdep = {
                b: [("bc", ((4 * b + oi) << 1) | kc, h)
                    for oi in range(4) for kc in range(2) for h in range(2)]
                for b in range(NBAND)
            }
            for t in range(WARM):
                for b in range(NBAND):
                    add(("mm", t, b), "pe", TMM,
                        mmdep[t] + bdep[b], 1, "mmps")
                    readers[("mm", t, b)] = [("ye", t, b)]
                    add(("ye", t, b), "dve", TYE, [("mm", t, b)], 2, "yq")
                    readers[("ye", t, b)] = [("yd", t, b)]
                    # early warm tiles store y on gpsimd (free after x
                    # casts), late ones on SP (free after W)
                    add(("yd", t, b), "q0" if t < 6 else "sp", TYD,
                        [("ye", t, b)], 3)

            engclk = {"dve": 0.0, "act": 0.0, "pe": 0.0, "sp": 0.0, "q0": 0.0}
            fifo = {p: [] for p in POOLS}
            emitted = []
            remaining = dict(ops)
            while remaining:
                best = None
                for key, (eng, cost, deps, pri, pool) in remaining.items():
                    if any(d not in ends for d in deps):
                        continue
                    start = max([engclk[eng]] + [ends[d] for d in deps])
                    if pool is not None:
                        n = len(fifo[pool])
                        if n >= POOLS[pool]:
                            victim = fifo[pool][n - POOLS[pool]]
                            rd = readers.get(victim, [])
                            if any(r not in ends for r in rd):
                                continue
                            start = max([start] + [ends[r] for r in rd])
                    if best is None or (start, pri) < best[0]:
                        best = ((start, pri), key)
                (start, _), key = best
                eng, cost, deps, pri, pool = remaining.pop(key)
                ends[key] = start + cost
                engclk[eng] = start + cost
                if pool is not None:
                    fifo[pool].append(key)
                emitted.append(key)

            # emit all warm ops (including DMA loads) in simulated order
            for k in emitted:
                if k[0] == "W":
                    e_wload(k[1], k[1] in q0_chunks)
                    continue
                if k[0] == "x":
                    e_xcast(k[1])
                    continue
                if k[0] == "cv":
                    e_conv(k[1], k[1] in q0_chunks)
                elif k[0] == "tx":
                    e_txbank(k[1], k[2])
                elif k[0] == "xc2":
                    e_txcopy(k[1], k[2])
                elif k[0] == "tb":
                    e_tbank(k[1], k[2])
                elif k[0] == "bc":
                    e_bcopy(k[1], k[2])
                elif k[0] == "xa":
                    e_xbarA(k[1])
                elif k[0] == "xb":
                    e_xbarB(k[1])
                elif k[0] == "x8":
                    e_x8(k[1])
                elif k[0] == "mm":
                    e_mm(k[1], k[2])
                elif k[0] == "ye":
                    t, b = k[1], k[2]
                    yq = yqpool.tile([P, BAND], f16, tag="yq")
                    ystw[(t, b)] = yq
                    e_yemit(t, b, yq, 0)
                elif k[0] == "yd":
                    t, b = k[1], k[2]
                    eng = nc.gpsimd if t < 6 else nc.sync
                    eng.dma_start(
                        y[t * P : (t + 1) * P, b * BAND : (b + 1) * BAND],
                        ystw.pop((t, b))[:],
                    )

            # ---------- steady phase (t10..12 already staged in warm) ----------
            for mt in range(WARM + LOOK, MT + LOOK):
                if mt < MT:
                    e_xcast(mt)
                    e_xbarA(mt)
                    e_xbarB(mt)
                    e_x8(mt)
                rt = mt - LOOK
                yst = ypool.tile([P, O_SH], f16, tag="yst")
                for b in range(NBAND):
                    e_mm(rt, b)
                    e_yemit(rt, b, yst, b * BAND)
                nc.sync.dma_start(y[rt * P : (rt + 1) * P, :], yst[:])

    nc.finalize()
    return nc


def _get_nc(apply_scale: bool):
    key = "scale" if apply_scale else "ones"
    if key not in _CACHE:
        _CACHE[key] = build(apply_scale)
    return _CACHE[key]


def kernel(x, weight, scale):
    global last_result
    x = np.ascontiguousarray(np.asarray(x, dtype=np.float32))
    weight = np.ascontiguousarray(np.asarray(weight, dtype=np.float32))
    scale = np.ascontiguousarray(np.asarray(scale, dtype=np.float32))
    apply_scale = not bool(np.all(scale == 1.0))
    nc = _get_nc(apply_scale)
    in_maps = [
        {
            "x": x,
            "weight": np.ascontiguousarray(weight[c * O_SH : (c + 1) * O_SH]),
            "scale": np.ascontiguousarray(scale[c * O_SH : (c + 1) * O_SH]),
        }
        for c in range(NCORES)
    ]
    res = run_bass_kernel_spmd(nc, in_maps, list(range(NCORES)))
    last_result = res
    return np.concatenate(
        [np.asarray(res.results[c]["y"]).astype(np.float32) for c in range(NCORES)],
        axis=1,
    )


if __name__ == "__main__":
    rng = np.random.default_rng(0)
    xv = rng.standard_normal((TOKENS, IN_F), dtype=np.float32)
    wv = rng.standard_normal((OUT_F, IN_F), dtype=np.float32)
    sv = np.ones(OUT_F, dtype=np.float32)
    yv = kernel(xv, wv, sv)
    print("out shape:", yv.shape, yv.dtype)
